# revision 20
# baseline (speedup 1.0000x reference)
"""BestRQ loss kernel for 8 Trainium2 NeuronCores (v3).

Math (exact reformulations of the reference):
  - loss = S0 - (sum_t m_t * L0[target_t]) / sum(m), with
    L0 = mask_emb @ W (shared logits row at every masked token) and
    S0 = logsumexp(L0).  Only masked tokens contribute.
  - target_t = argmax_n score_tn, score_tn = proj_t . emb_n - 0.5|emb_n|^2.
  - The kernel computes beta-scaled scores directly (beta folded into the
    matmul lhs), per 1024-code block g:
        nb_g   = -max_n beta*score          (DVE reduce, negate=True)
        psum  += beta*delta*L0              (K=1 accum matmul vs row17)
        vsum_g = sum_n exp(beta*v + nb_g)   (ACT exp, bias=nb_g, accum)
    then vtot = sum_g vsum_g * exp(-nb_g - max_g(-nb_g)) ~= exp(beta*delta*
    L0[argmax]); the host takes ln(vtot) (keeps Ln out of the hot loop so
    only one ACT table set is ever loaded mid-kernel).
  - 4096 masked tokens -> 4 tiles x 128 per core; the <=128 leftovers are
    replicated on every core as a "tail" tile where each core scores only
    its own 1/8 of the codebook (per-core block-rotated codebook; argmax is
    column-order invariant) and the host combines the per-core partials.
  - W is shipped fp8e4 and streamed once to build the delta*L0 row (16
    matmuls into col-group-packed PSUM rows, M=32 replication so the row
    escapes PSUM in one full-width ACT copy); S0 comes from that row
    reshaped to [128,64].
"""

import math

import numpy as np

try:
    import concourse.bass as bass  # noqa: F401
except ImportError:  # pragma: no cover
    import sys

    sys.path.insert(0, "/opt/trn_rl_repo")
    import concourse.bass as bass  # noqa: F401

import concourse.mybir as mybir
from concourse import bacc, bass_utils, masks
from concourse.tile import TileContext

F32 = mybir.dt.float32
BF16 = mybir.dt.bfloat16
FP8 = mybir.dt.float8e4

B, T, D, E, N = 16, 512, 256, 16, 8192
NCORES = 8
EPS = 1e-5
DELTA = 1e-2
BETA = 2000.0
NBLK = 8          # 1024-code blocks
BLK = N // NBLK

_CACHE = {}


def _build_bass(NT, use_tail):
    nc = bacc.Bacc(
        "TRN2", target_bir_lowering=False, debug=False, num_devices=NCORES
    )
    NLN = NT + (1 if use_tail else 0)
    xsm = nc.dram_tensor("xsm", [128, NT, D], F32, kind="ExternalInput")
    xst = nc.dram_tensor("xst", [128, D], F32, kind="ExternalInput")
    embb = nc.dram_tensor("embb", [E, N], BF16, kind="ExternalInput")
    qrow = nc.dram_tensor("qrow", [1, N], BF16, kind="ExternalInput")
    ppb = nc.dram_tensor("ppb", [128, 2, E], BF16, kind="ExternalInput")
    b0t = nc.dram_tensor("b0t", [E, 1], F32, kind="ExternalInput")
    mk2 = nc.dram_tensor("mk2", [128, 2, 32], FP8, kind="ExternalInput")
    idin = nc.dram_tensor("idin", [128, 128], BF16, kind="ExternalInput")
    wb = nc.dram_tensor("wb", [NBLK, 128, 2, 1024], FP8, kind="ExternalInput")
    out = nc.dram_tensor("out", [128, NT + 3], F32, kind="ExternalOutput")

    AX = mybir.AxisListType.X
    OP = mybir.AluOpType
    AF = mybir.ActivationFunctionType

    with TileContext(nc) as tc:
        with (
            tc.tile_pool(name="cst", bufs=1) as cst,
            tc.tile_pool(name="xsp", bufs=1) as xsp,
            tc.tile_pool(name="wp", bufs=1) as wp,
            tc.tile_pool(name="lnp", bufs=2) as lnp,
            tc.tile_pool(name="lhp", bufs=NLN) as lhp,
            tc.tile_pool(name="smp", bufs=2) as smp,
            tc.tile_pool(name="ps", bufs=3, space="PSUM") as ps,
            tc.tile_pool(name="psm", bufs=1, space="PSUM") as psm,
        ):
            # ---------------- constants / big DMAs ----------------
            em17 = cst.tile([17, N], BF16)
            row17 = cst.tile([1, N], BF16)
            xall = xsp.tile([128, NT, D], F32)
            nc.sync.dma_start(xall[:], xsm[:, :, :])
            xtail = xsp.tile([128, D], F32)
            if use_tail:
                nc.sync.dma_start(xtail[:], xst[:, :])
            mk = cst.tile([128, 2, 32], FP8)
            nc.sync.dma_start(mk[:], mk2[:, :, :])
            wres = []
            for q in range(4):
                wt = wp.tile([128, 2, 2, 1024], FP8, tag="wt", name="wt",
                             bufs=4)
                nc.sync.dma_start(wt[:], wb[2 * q:2 * q + 2, :, :, :])
                wres.append(wt)
            nc.sync.dma_start(em17[0:16, :], embb[:, :])
            nc.sync.dma_start(em17[16:17, :], qrow[:, :])
            ident = cst.tile([128, 128], BF16)
            nc.sync.dma_start(ident[:], idin[:, :])
            pp = cst.tile([128, 2, E], BF16)
            nc.sync.dma_start(pp[:], ppb[:, :, :])
            b0 = cst.tile([E, 1], F32)
            nc.sync.dma_start(b0[:], b0t[:, :])

            onesb = cst.tile([1, 128], BF16)     # beta row for K=1 accum
            nc.vector.memset(onesb[:], BETA)

            epsb = cst.tile([128, 1], F32)
            nc.vector.memset(epsb[:], EPS)

            etr = cst.tile([128, BLK], BF16)       # exp trash output
            dl0rep = cst.tile([128, 1024], BF16)   # delta*L0, 32x-replicated
            s0t = cst.tile([128, 64], BF16)
            s0acc = cst.tile([128, 1], F32)
            vt_all = cst.tile([128, NT], F32)
            mvall = cst.tile([128, 2 * NLN], F32)
            lnv_all = cst.tile([128, NLN], F32)
            rstd_all = cst.tile([128, NLN], F32)

            # ---------------- LN stats, batched by ACT table set ----------
            for i in range(NLN):
                x_t = xall[:, i, :] if i < NT else xtail[:]
                st6 = lnp.tile([128, 6], F32, tag="st6")
                nc.vector.bn_stats(st6[:], x_t)
                nc.vector.bn_aggr(mvall[:, 2 * i:2 * i + 2], st6[:])
            for i in range(NLN):
                nc.scalar.activation(
                    lnv_all[:, i:i + 1], mvall[:, 2 * i + 1:2 * i + 2],
                    AF.Ln, bias=epsb[:],
                )
            for i in range(NLN):
                nc.scalar.activation(
                    rstd_all[:, i:i + 1], lnv_all[:, i:i + 1],
                    AF.Exp, scale=-0.5,
                )

            def ln_tile(i, early=False):
                """z -> zT -> proj -> beta-scaled lhs [17,128] for tile i."""
                x_t = xall[:, i, :] if i < NT else xtail[:]
                z = lnp.tile([128, D], BF16, tag="z")
                nc.vector.tensor_scalar(
                    z[:], x_t, mvall[:, 2 * i:2 * i + 1],
                    rstd_all[:, i:i + 1], op0=OP.subtract, op1=OP.mult,
                )
                pool, tg = (ps, "pair") if early else (psm, "misc")
                mtz = pool.tile([128, 1024], F32, tag=tg, name="mtz")
                ztp = mtz[:].bitcast(BF16)[:, 0:256]
                for kc in range(2):
                    nc.tensor.transpose(
                        ztp[:, kc * 128:(kc + 1) * 128],
                        z[:, kc * 128:(kc + 1) * 128], ident[:],
                    )
                zt = lnp.tile([128, 2, 128], BF16, tag="ztsb")
                nc.scalar.activation(zt[:, 0, :], ztp[:, 0:128], AF.Copy)
                nc.scalar.activation(zt[:, 1, :], ztp[:, 128:256], AF.Copy)
                mtp = pool.tile([128, 1024], F32, tag=tg, name="mtp")
                ppj = mtp[0:16, 0:128]
                for kc in range(2):
                    nc.tensor.matmul(
                        ppj, pp[:, kc, :], zt[:, kc, :],
                        start=(kc == 0), stop=(kc == 1),
                    )
                lhs = lhp.tile([17, 128], BF16, tag="lhs")
                nc.vector.memset(lhs[:], BETA)   # row16 = beta (q-row coeff)
                nc.vector.tensor_scalar(
                    lhs[0:16, :], ppj, b0[:], BETA, op0=OP.add, op1=OP.mult
                )
                return lhs

            def psl_batch(b):
                """fp8 W chunks 4b..4b+3 -> delta*L0 row cols [4096b:...]."""
                psl = psm.tile([128, 1024], F32, tag="misc", name="psl")[:]
                for cg in range(4):
                    g = 4 * b + cg
                    for h in range(2):
                        hs = slice(h * 512, (h + 1) * 512)
                        for dc in range(2):
                            nc.tensor.matmul(
                                psl[32 * cg:32 * cg + 32, hs],
                                mk[:, dc, :], wres[g // 2][:, g % 2, dc, hs],
                                start=(dc == 0), stop=(dc == 1),
                                tile_position=(0, 32 * cg),
                            )
                nc.scalar.activation(dl0rep[:, :], psl, AF.Copy, scale=DELTA)
                src = dl0rep[:].rearrange("(c s) j -> c s j", s=32)[:, 0:1, :]
                nc.sync.dma_start(row17[0:1, b * 4096:(b + 1) * 4096], src)

            def mm_score(lhs, g, pool, tg):
                pt = pool.tile([128, BLK], F32, tag=tg, name="pt")
                for h in range(2):
                    hs = slice(g * BLK + h * 512, g * BLK + (h + 1) * 512)
                    nc.tensor.matmul(
                        pt[:, h * 512:(h + 1) * 512], lhs[0:17, :],
                        em17[0:17, hs], start=True, stop=True,
                    )
                return pt

            def block_tail(pt, g, nb, vsum):
                for h in range(2):
                    hs = slice(g * BLK + h * 512, g * BLK + (h + 1) * 512)
                    nc.tensor.matmul(
                        pt[:, h * 512:(h + 1) * 512], onesb[:],
                        row17[0:1, hs], start=False, stop=True,
                        skip_group_check=True,
                    )
                nc.scalar.activation(
                    etr[:], pt[:], AF.Exp, bias=nb[:, g:g + 1],
                    accum_out=vsum[:, g:g + 1],
                )

            def do_tile(lhs, nblk, nb, vsum, lag=3):
                """Software-pipelined: MMdelta/EXP trail MM_s by `lag`
                blocks so no PE instruction waits on an in-flight DVE max.
                Every 4th block borrows the misc PSUM ring for a 4-deep
                effective pipeline."""
                pend = []
                for g in range(nblk):
                    if g % 4 == 3:
                        pt = mm_score(lhs, g, psm, "misc")
                    else:
                        pt = mm_score(lhs, g, ps, "pair")
                    nc.vector.tensor_reduce(
                        nb[:, g:g + 1], pt[:], axis=AX, op=OP.max, negate=True
                    )
                    pend.append((pt, g))
                    if len(pend) > lag:
                        block_tail(*pend.pop(0), nb, vsum)
                for item in pend:
                    block_tail(*item, nb, vsum)

            def combine_tile(i, nb, vsum):
                tmin = smp.tile([128, 1], F32, tag="tmin")
                nc.vector.tensor_reduce(tmin[:], nb[:], axis=AX, op=OP.min)
                wg = smp.tile([128, NBLK], F32, tag="wg")
                nc.scalar.activation(
                    wg[:], nb[:], AF.Exp, scale=-1.0, bias=tmin[:]
                )
                vd = smp.tile([128, NBLK], F32, tag="vd")
                nc.vector.tensor_tensor(vd[:], vsum[:], wg[:], op=OP.mult)
                nc.vector.tensor_reduce(
                    vt_all[:, i:i + 1], vd[:], axis=AX, op=OP.add
                )

            def tile_state():
                nb = smp.tile([128, NBLK], F32, tag="nb")
                vsum = smp.tile([128, NBLK], F32, tag="vsum")
                return nb, vsum

            # ---------------- emission schedule ----------------
            psl_batch(0)                       # row17 cols 0:4096
            lhs_all = {}
            lhs_all[0] = ln_tile(0, early=True)
            if NLN > 1:
                lhs_all[1] = ln_tile(1, early=True)
            psl_batch(1)                       # row17 cols 4096:8192

            st0 = tile_state()
            do_tile(lhs_all[0], NBLK, st0[0], st0[1])
            combine_tile(0, st0[0], st0[1])

            for i in range(2, NLN):
                lhs_all[i] = ln_tile(i)

            # S0 partials: row17 -> [128,64] -> exp-accum
            nc.sync.dma_start(s0t[:], row17[0:1, :])
            nc.scalar.activation(
                etr[:, 0:64], s0t[:], AF.Exp, scale=1.0 / DELTA,
                accum_out=s0acc[:],
            )

            for i in range(1, NT):
                sti = tile_state()
                do_tile(lhs_all[i], NBLK, sti[0], sti[1])
                combine_tile(i, sti[0], sti[1])
                if i == 1 and use_tail:
                    stt = tile_state()
                    do_tile(lhs_all[NT], 1, stt[0], stt[1])
                    nc.sync.dma_start(out[:, NT + 1:NT + 2], stt[0][:, 0:1])
                    nc.sync.dma_start(out[:, NT + 2:NT + 3], stt[1][:, 0:1])
            if use_tail and NT == 1:
                stt = tile_state()
                do_tile(lhs_all[NT], 1, stt[0], stt[1])
                nc.sync.dma_start(out[:, NT + 1:NT + 2], stt[0][:, 0:1])
                nc.sync.dma_start(out[:, NT + 2:NT + 3], stt[1][:, 0:1])

            # outputs
            nc.sync.dma_start(out[:, 0:NT], vt_all[:])
            nc.sync.dma_start(out[:, NT:NT + 1], s0acc[:])

    nc.finalize()
    return nc


def _prep_in_maps(xs, pad_mask, masked_masks, ln_gamma, ln_beta, projection,
                  embeddings, top_n_out, mask_emb):
    import ml_dtypes

    xsf = np.ascontiguousarray(np.asarray(xs, np.float32).reshape(B * T, D))
    pmf = np.asarray(pad_mask).reshape(-1).astype(bool)
    mmf = np.asarray(masked_masks).reshape(-1).astype(bool)
    gamma = np.asarray(ln_gamma, np.float32)
    beta = np.asarray(ln_beta, np.float32)
    proj = np.asarray(projection, np.float32)
    emb = np.asarray(embeddings, np.float32)[0]          # [E, N]
    wmat = np.asarray(top_n_out, np.float32)[0]          # [D, N]
    maske = np.asarray(mask_emb, np.float32)

    sel = np.nonzero(pmf & mmf)[0]
    n = len(sel)
    NT = max(1, -(-max(n - 128, 1) // (NCORES * 128)))
    nmain = min(n, NCORES * 128 * NT)
    L = n - nmain
    assert L <= 128, f"tail overflow: {L}"
    use_tail = L > 0

    main_idx = sel[:nmain]
    xs_cores, m_cores = [], []
    for c in range(NCORES):
        idx = main_idx[c * 128 * NT:(c + 1) * 128 * NT]
        k = len(idx)
        xc = np.zeros((NT * 128, D), np.float32)
        mc = np.zeros((NT * 128,), np.float32)
        if k:
            xc[:k] = xsf[idx]
            mc[:k] = 1.0
        xs_cores.append(
            np.ascontiguousarray(xc.reshape(NT, 128, D).transpose(1, 0, 2))
        )
        m_cores.append(
            np.ascontiguousarray(mc.reshape(NT, 128).transpose(1, 0))
        )

    xt = np.zeros((128, D), np.float32)
    if use_tail:
        xt[:L] = xsf[sel[nmain:]]

    # gamma folded into projection, beta into the projected bias (host fold)
    ppf = (gamma[:, None] * proj).astype(np.float32)     # [D, E]
    b0 = (beta @ proj).astype(np.float32).reshape(E, 1)  # [E, 1]
    ppb = np.ascontiguousarray(
        ppf.reshape(2, 128, E).transpose(1, 0, 2)
    ).astype(ml_dtypes.bfloat16)

    # mk2[p, dc, j] = maske[dc*128 + p], replicated over 32 cols (M=32)
    mk2 = np.ascontiguousarray(
        np.repeat(maske.reshape(2, 128).T[:, :, None], 32, axis=2)
    ).astype(ml_dtypes.float8_e4m3fn)

    in_maps = []
    for c in range(NCORES):
        perm = np.roll(np.arange(N), -c * 1024)
        embP = emb[:, perm]
        wP = wmat[:, perm]
        qrowP = (-0.5 * np.sum(embP.astype(np.float64) ** 2, axis=0)).astype(
            np.float32
        ).reshape(1, N)
        wbP = np.ascontiguousarray(
            wP.reshape(2, 128, NBLK, 1024).transpose(2, 1, 0, 3)
        )
        in_maps.append({
            "xsm": xs_cores[c],
            "xst": xt,
            "idin": np.eye(128, dtype=np.float32).astype(ml_dtypes.bfloat16),
            "embb": np.ascontiguousarray(embP).astype(ml_dtypes.bfloat16),
            "qrow": qrowP.astype(ml_dtypes.bfloat16),
            "ppb": ppb,
            "b0t": b0,
            "mk2": mk2,
            "wb": wbP.astype(ml_dtypes.float8_e4m3fn),
        })
    return in_maps, (NT, use_tail, n, L, m_cores)


def kernel(**inputs) -> np.ndarray:
    in_maps, (NT, use_tail, n, L, m_cores) = _prep_in_maps(**inputs)
    key = (NT, use_tail)
    if key not in _CACHE:
        _CACHE[key] = _build_bass(NT, use_tail)
        _CACHE["nc"] = _CACHE[key]
    nc = _CACHE[key]
    res = bass_utils.run_bass_kernel_spmd(
        nc, in_maps, core_ids=list(range(NCORES))
    )
    num = 0.0
    s0sum = None
    bmt = np.zeros((NCORES, 128), np.float64)
    vst = np.zeros((NCORES, 128), np.float64)
    for c, r in enumerate(res.results):
        o = np.asarray(r["out"], np.float64).reshape(128, NT + 3)
        m = m_cores[c].astype(np.float64)                 # [128, NT]
        num += float((m * np.log(o[:, 0:NT])).sum())
        s0sum = float(o[:, NT].sum())
        bmt[c] = -o[:, NT + 1]                            # beta*bmax
        vst[c] = o[:, NT + 2]
    if L > 0:
        gm = bmt.max(axis=0)
        w = np.exp(bmt - gm[None, :])
        vtot = (vst * w).sum(axis=0)
        num += float(np.log(vtot[:L]).sum())
    s0 = math.log(s0sum)
    loss = np.float32(s0 - num / (BETA * DELTA) / n)
    return np.asarray(loss, np.float32)


# revision 21
# speedup vs baseline: 1.0213x; 1.0213x over previous
"""BestRQ loss kernel for 8 Trainium2 NeuronCores (v3).

Math (exact reformulations of the reference):
  - loss = S0 - (sum_t m_t * L0[target_t]) / sum(m), with
    L0 = mask_emb @ W (shared logits row at every masked token) and
    S0 = logsumexp(L0).  Only masked tokens contribute.
  - target_t = argmax_n score_tn, score_tn = proj_t . emb_n - 0.5|emb_n|^2.
  - The kernel computes beta-scaled scores directly (beta folded into the
    matmul lhs), per 1024-code block g:
        nb_g   = -max_n beta*score          (DVE reduce, negate=True)
        psum  += beta*delta*L0              (K=1 accum matmul vs row17)
        vsum_g = sum_n exp(beta*v + nb_g)   (ACT exp, bias=nb_g, accum)
    then vtot = sum_g vsum_g * exp(-nb_g - max_g(-nb_g)) ~= exp(beta*delta*
    L0[argmax]); the host takes ln(vtot) (keeps Ln out of the hot loop so
    only one ACT table set is ever loaded mid-kernel).
  - 4096 masked tokens -> 4 tiles x 128 per core; the <=128 leftovers are
    replicated on every core as a "tail" tile where each core scores only
    its own 1/8 of the codebook (per-core block-rotated codebook; argmax is
    column-order invariant) and the host combines the per-core partials.
  - W is shipped fp8e4 and streamed once to build the delta*L0 row (16
    matmuls into col-group-packed PSUM rows, M=32 replication so the row
    escapes PSUM in one full-width ACT copy); S0 comes from that row
    reshaped to [128,64].
"""

import math

import numpy as np

try:
    import concourse.bass as bass  # noqa: F401
except ImportError:  # pragma: no cover
    import sys

    sys.path.insert(0, "/opt/trn_rl_repo")
    import concourse.bass as bass  # noqa: F401

import concourse.mybir as mybir
from concourse import bacc, bass_utils, masks
from concourse.tile import TileContext

F32 = mybir.dt.float32
BF16 = mybir.dt.bfloat16
FP8 = mybir.dt.float8e4

B, T, D, E, N = 16, 512, 256, 16, 8192
NCORES = 8
EPS = 1e-5
DELTA = 1e-2
BETA = 2000.0
NBLK = 8          # 1024-code blocks
BLK = N // NBLK

_CACHE = {}


def _build_bass(NT, use_tail):
    nc = bacc.Bacc(
        "TRN2", target_bir_lowering=False, debug=False, num_devices=NCORES
    )
    NLN = NT + (1 if use_tail else 0)
    xsm = nc.dram_tensor("xsm", [128, NT, D], F32, kind="ExternalInput")
    xst = nc.dram_tensor("xst", [128, D], F32, kind="ExternalInput")
    embb = nc.dram_tensor("embb", [E, N], BF16, kind="ExternalInput")
    qrow = nc.dram_tensor("qrow", [1, N], BF16, kind="ExternalInput")
    ppb = nc.dram_tensor("ppb", [128, 2, E], BF16, kind="ExternalInput")
    b0t = nc.dram_tensor("b0t", [E, 1], F32, kind="ExternalInput")
    mk2 = nc.dram_tensor("mk2", [128, 2, 32], FP8, kind="ExternalInput")
    idin = nc.dram_tensor("idin", [128, 128], BF16, kind="ExternalInput")
    wb = nc.dram_tensor("wb", [NBLK, 128, 2, 1024], FP8, kind="ExternalInput")
    out = nc.dram_tensor("out", [128, NT + 3], F32, kind="ExternalOutput")

    AX = mybir.AxisListType.X
    OP = mybir.AluOpType
    AF = mybir.ActivationFunctionType

    with TileContext(nc) as tc:
        with (
            tc.tile_pool(name="cst", bufs=1) as cst,
            tc.tile_pool(name="xsp", bufs=1) as xsp,
            tc.tile_pool(name="wp", bufs=1) as wp,
            tc.tile_pool(name="lnp", bufs=2) as lnp,
            tc.tile_pool(name="lhp", bufs=NLN) as lhp,
            tc.tile_pool(name="smp", bufs=2) as smp,
            tc.tile_pool(name="ps", bufs=3, space="PSUM") as ps,
            tc.tile_pool(name="psm", bufs=1, space="PSUM") as psm,
        ):
            # ---------------- constants / big DMAs ----------------
            em17 = cst.tile([17, N], BF16)
            row17 = cst.tile([1, N], BF16)
            mk = cst.tile([128, 2, 32], FP8)
            nc.sync.dma_start(mk[:], mk2[:, :, :])
            xall = xsp.tile([128, NT, D], F32)
            xtail = xsp.tile([128, D], F32)
            wres = []
            for g in range(NBLK):
                wt = wp.tile([128, 2, 1024], FP8, tag="wt", name="wt",
                             bufs=NBLK)
                nc.sync.dma_start(wt[:], wb[g, :, :, :])
                wres.append(wt)
                if g == 1:
                    nc.sync.dma_start(xall[:], xsm[:, :, :])
                    if use_tail:
                        nc.sync.dma_start(xtail[:], xst[:, :])
            nc.sync.dma_start(em17[0:16, :], embb[:, :])
            nc.sync.dma_start(em17[16:17, :], qrow[:, :])
            ident = cst.tile([128, 128], BF16)
            nc.sync.dma_start(ident[:], idin[:, :])
            pp = cst.tile([128, 2, E], BF16)
            nc.sync.dma_start(pp[:], ppb[:, :, :])
            b0 = cst.tile([E, 1], F32)
            nc.sync.dma_start(b0[:], b0t[:, :])

            onesb = cst.tile([1, 128], BF16)     # beta row for K=1 accum
            nc.vector.memset(onesb[:], BETA)

            epsb = cst.tile([128, 1], F32)
            nc.vector.memset(epsb[:], EPS)

            etr = cst.tile([128, BLK], BF16)       # exp trash output
            dl0rep = cst.tile([128, 1024], BF16)   # delta*L0, 32x-replicated
            s0t = cst.tile([128, 64], BF16)
            s0acc = cst.tile([128, 1], F32)
            vt_all = cst.tile([128, NT], F32)
            mvall = cst.tile([128, 2 * NLN], F32)
            lnv_all = cst.tile([128, NLN], F32)
            rstd_all = cst.tile([128, NLN], F32)

            # ---------------- LN stats, batched by ACT table set ----------
            for i in range(NLN):
                x_t = xall[:, i, :] if i < NT else xtail[:]
                st6 = lnp.tile([128, 6], F32, tag="st6")
                nc.vector.bn_stats(st6[:], x_t)
                nc.vector.bn_aggr(mvall[:, 2 * i:2 * i + 2], st6[:])
            for i in range(NLN):
                nc.scalar.activation(
                    lnv_all[:, i:i + 1], mvall[:, 2 * i + 1:2 * i + 2],
                    AF.Ln, bias=epsb[:],
                )
            for i in range(NLN):
                nc.scalar.activation(
                    rstd_all[:, i:i + 1], lnv_all[:, i:i + 1],
                    AF.Exp, scale=-0.5,
                )

            def ln_tile(i, early=False):
                """z -> zT -> proj -> beta-scaled lhs [17,128] for tile i."""
                x_t = xall[:, i, :] if i < NT else xtail[:]
                z = lnp.tile([128, D], BF16, tag="z")
                nc.vector.tensor_scalar(
                    z[:], x_t, mvall[:, 2 * i:2 * i + 1],
                    rstd_all[:, i:i + 1], op0=OP.subtract, op1=OP.mult,
                )
                pool, tg = (ps, "pair") if early else (psm, "misc")
                mtz = pool.tile([128, 1024], F32, tag=tg, name="mtz")
                ztp = mtz[:].bitcast(BF16)[:, 0:256]
                for kc in range(2):
                    nc.tensor.transpose(
                        ztp[:, kc * 128:(kc + 1) * 128],
                        z[:, kc * 128:(kc + 1) * 128], ident[:],
                    )
                zt = lnp.tile([128, 2, 128], BF16, tag="ztsb")
                nc.scalar.activation(zt[:, 0, :], ztp[:, 0:128], AF.Copy)
                nc.scalar.activation(zt[:, 1, :], ztp[:, 128:256], AF.Copy)
                mtp = pool.tile([128, 1024], F32, tag=tg, name="mtp")
                ppj = mtp[0:16, 0:128]
                for kc in range(2):
                    nc.tensor.matmul(
                        ppj, pp[:, kc, :], zt[:, kc, :],
                        start=(kc == 0), stop=(kc == 1),
                    )
                lhs = lhp.tile([17, 128], BF16, tag="lhs")
                nc.vector.memset(lhs[:], BETA)   # row16 = beta (q-row coeff)
                nc.vector.tensor_scalar(
                    lhs[0:16, :], ppj, b0[:], BETA, op0=OP.add, op1=OP.mult
                )
                return lhs

            def psl_batch(b):
                """fp8 W chunks 4b..4b+3 -> delta*L0 row cols [4096b:...]."""
                psl = psm.tile([128, 1024], F32, tag="misc", name="psl")[:]
                for cg in range(4):
                    g = 4 * b + cg
                    for h in range(2):
                        hs = slice(h * 512, (h + 1) * 512)
                        for dc in range(2):
                            nc.tensor.matmul(
                                psl[32 * cg:32 * cg + 32, hs],
                                mk[:, dc, :], wres[g][:, dc, hs],
                                start=(dc == 0), stop=(dc == 1),
                                tile_position=(0, 32 * cg),
                            )
                nc.scalar.activation(dl0rep[:, :], psl, AF.Copy, scale=DELTA)
                src = dl0rep[:].rearrange("(c s) j -> c s j", s=32)[:, 0:1, :]
                nc.sync.dma_start(row17[0:1, b * 4096:(b + 1) * 4096], src)

            def mm_score(lhs, g, pool, tg):
                pt = pool.tile([128, BLK], F32, tag=tg, name="pt")
                for h in range(2):
                    hs = slice(g * BLK + h * 512, g * BLK + (h + 1) * 512)
                    nc.tensor.matmul(
                        pt[:, h * 512:(h + 1) * 512], lhs[0:17, :],
                        em17[0:17, hs], start=True, stop=True,
                    )
                return pt

            def block_tail(pt, g, nb, vsum):
                for h in range(2):
                    hs = slice(g * BLK + h * 512, g * BLK + (h + 1) * 512)
                    nc.tensor.matmul(
                        pt[:, h * 512:(h + 1) * 512], onesb[:],
                        row17[0:1, hs], start=False, stop=True,
                        skip_group_check=True,
                    )
                nc.scalar.activation(
                    etr[:], pt[:], AF.Exp, bias=nb[:, g:g + 1],
                    accum_out=vsum[:, g:g + 1],
                )

            def do_tile(lhs, nblk, nb, vsum, lag=3):
                """Software-pipelined: MMdelta/EXP trail MM_s by `lag`
                blocks so no PE instruction waits on an in-flight DVE max.
                Every 4th block borrows the misc PSUM ring for a 4-deep
                effective pipeline."""
                pend = []
                for g in range(nblk):
                    if g % 4 == 3:
                        pt = mm_score(lhs, g, psm, "misc")
                    else:
                        pt = mm_score(lhs, g, ps, "pair")
                    nc.vector.tensor_reduce(
                        nb[:, g:g + 1], pt[:], axis=AX, op=OP.max, negate=True
                    )
                    pend.append((pt, g))
                    if len(pend) > lag:
                        block_tail(*pend.pop(0), nb, vsum)
                for item in pend:
                    block_tail(*item, nb, vsum)

            def combine_tile(i, nb, vsum):
                tmin = smp.tile([128, 1], F32, tag="tmin")
                nc.vector.tensor_reduce(tmin[:], nb[:], axis=AX, op=OP.min)
                wg = smp.tile([128, NBLK], F32, tag="wg")
                nc.scalar.activation(
                    wg[:], nb[:], AF.Exp, scale=-1.0, bias=tmin[:]
                )
                vd = smp.tile([128, NBLK], F32, tag="vd")
                nc.vector.tensor_tensor(vd[:], vsum[:], wg[:], op=OP.mult)
                nc.vector.tensor_reduce(
                    vt_all[:, i:i + 1], vd[:], axis=AX, op=OP.add
                )

            def tile_state():
                nb = smp.tile([128, NBLK], F32, tag="nb")
                vsum = smp.tile([128, NBLK], F32, tag="vsum")
                return nb, vsum

            # ---------------- emission schedule ----------------
            psl_batch(0)                       # row17 cols 0:4096
            lhs_all = {}
            lhs_all[0] = ln_tile(0, early=True)
            if NLN > 1:
                lhs_all[1] = ln_tile(1, early=True)
            psl_batch(1)                       # row17 cols 4096:8192

            st0 = tile_state()
            do_tile(lhs_all[0], NBLK, st0[0], st0[1])
            combine_tile(0, st0[0], st0[1])

            for i in range(2, NLN):
                lhs_all[i] = ln_tile(i, early=True)

            # S0 partials: row17 -> [128,64] -> exp-accum
            nc.sync.dma_start(s0t[:], row17[0:1, :])
            nc.scalar.activation(
                etr[:, 0:64], s0t[:], AF.Exp, scale=1.0 / DELTA,
                accum_out=s0acc[:],
            )

            for i in range(1, NT):
                sti = tile_state()
                do_tile(lhs_all[i], NBLK, sti[0], sti[1])
                combine_tile(i, sti[0], sti[1])
                if i == 1 and use_tail:
                    stt = tile_state()
                    do_tile(lhs_all[NT], 1, stt[0], stt[1])
                    nc.sync.dma_start(out[:, NT + 1:NT + 2], stt[0][:, 0:1])
                    nc.sync.dma_start(out[:, NT + 2:NT + 3], stt[1][:, 0:1])
            if use_tail and NT == 1:
                stt = tile_state()
                do_tile(lhs_all[NT], 1, stt[0], stt[1])
                nc.sync.dma_start(out[:, NT + 1:NT + 2], stt[0][:, 0:1])
                nc.sync.dma_start(out[:, NT + 2:NT + 3], stt[1][:, 0:1])

            # outputs
            nc.sync.dma_start(out[:, 0:NT], vt_all[:])
            nc.sync.dma_start(out[:, NT:NT + 1], s0acc[:])

    nc.finalize()
    return nc


def _prep_in_maps(xs, pad_mask, masked_masks, ln_gamma, ln_beta, projection,
                  embeddings, top_n_out, mask_emb):
    import ml_dtypes

    xsf = np.ascontiguousarray(np.asarray(xs, np.float32).reshape(B * T, D))
    pmf = np.asarray(pad_mask).reshape(-1).astype(bool)
    mmf = np.asarray(masked_masks).reshape(-1).astype(bool)
    gamma = np.asarray(ln_gamma, np.float32)
    beta = np.asarray(ln_beta, np.float32)
    proj = np.asarray(projection, np.float32)
    emb = np.asarray(embeddings, np.float32)[0]          # [E, N]
    wmat = np.asarray(top_n_out, np.float32)[0]          # [D, N]
    maske = np.asarray(mask_emb, np.float32)

    sel = np.nonzero(pmf & mmf)[0]
    n = len(sel)
    NT = max(1, -(-max(n - 128, 1) // (NCORES * 128)))
    nmain = min(n, NCORES * 128 * NT)
    L = n - nmain
    assert L <= 128, f"tail overflow: {L}"
    use_tail = L > 0

    main_idx = sel[:nmain]
    xs_cores, m_cores = [], []
    for c in range(NCORES):
        idx = main_idx[c * 128 * NT:(c + 1) * 128 * NT]
        k = len(idx)
        xc = np.zeros((NT * 128, D), np.float32)
        mc = np.zeros((NT * 128,), np.float32)
        if k:
            xc[:k] = xsf[idx]
            mc[:k] = 1.0
        xs_cores.append(
            np.ascontiguousarray(xc.reshape(NT, 128, D).transpose(1, 0, 2))
        )
        m_cores.append(
            np.ascontiguousarray(mc.reshape(NT, 128).transpose(1, 0))
        )

    xt = np.zeros((128, D), np.float32)
    if use_tail:
        xt[:L] = xsf[sel[nmain:]]

    # gamma folded into projection, beta into the projected bias (host fold)
    ppf = (gamma[:, None] * proj).astype(np.float32)     # [D, E]
    b0 = (beta @ proj).astype(np.float32).reshape(E, 1)  # [E, 1]
    ppb = np.ascontiguousarray(
        ppf.reshape(2, 128, E).transpose(1, 0, 2)
    ).astype(ml_dtypes.bfloat16)

    # mk2[p, dc, j] = maske[dc*128 + p], replicated over 32 cols (M=32)
    mk2 = np.ascontiguousarray(
        np.repeat(maske.reshape(2, 128).T[:, :, None], 32, axis=2)
    ).astype(ml_dtypes.float8_e4m3fn)

    in_maps = []
    for c in range(NCORES):
        perm = np.roll(np.arange(N), -c * 1024)
        embP = emb[:, perm]
        wP = wmat[:, perm]
        qrowP = (-0.5 * np.sum(embP.astype(np.float64) ** 2, axis=0)).astype(
            np.float32
        ).reshape(1, N)
        wbP = np.ascontiguousarray(
            wP.reshape(2, 128, NBLK, 1024).transpose(2, 1, 0, 3)
        )
        in_maps.append({
            "xsm": xs_cores[c],
            "xst": xt,
            "idin": np.eye(128, dtype=np.float32).astype(ml_dtypes.bfloat16),
            "embb": np.ascontiguousarray(embP).astype(ml_dtypes.bfloat16),
            "qrow": qrowP.astype(ml_dtypes.bfloat16),
            "ppb": ppb,
            "b0t": b0,
            "mk2": mk2,
            "wb": wbP.astype(ml_dtypes.float8_e4m3fn),
        })
    return in_maps, (NT, use_tail, n, L, m_cores)


def kernel(**inputs) -> np.ndarray:
    in_maps, (NT, use_tail, n, L, m_cores) = _prep_in_maps(**inputs)
    key = (NT, use_tail)
    if key not in _CACHE:
        _CACHE[key] = _build_bass(NT, use_tail)
        _CACHE["nc"] = _CACHE[key]
    nc = _CACHE[key]
    res = bass_utils.run_bass_kernel_spmd(
        nc, in_maps, core_ids=list(range(NCORES))
    )
    num = 0.0
    s0sum = None
    bmt = np.zeros((NCORES, 128), np.float64)
    vst = np.zeros((NCORES, 128), np.float64)
    for c, r in enumerate(res.results):
        o = np.asarray(r["out"], np.float64).reshape(128, NT + 3)
        m = m_cores[c].astype(np.float64)                 # [128, NT]
        num += float((m * np.log(o[:, 0:NT])).sum())
        s0sum = float(o[:, NT].sum())
        bmt[c] = -o[:, NT + 1]                            # beta*bmax
        vst[c] = o[:, NT + 2]
    if L > 0:
        gm = bmt.max(axis=0)
        w = np.exp(bmt - gm[None, :])
        vtot = (vst * w).sum(axis=0)
        num += float(np.log(vtot[:L]).sum())
    s0 = math.log(s0sum)
    loss = np.float32(s0 - num / (BETA * DELTA) / n)
    return np.asarray(loss, np.float32)


# revision 22
# speedup vs baseline: 1.0514x; 1.0294x over previous
"""BestRQ loss kernel for 8 Trainium2 NeuronCores (v3).

Math (exact reformulations of the reference):
  - loss = S0 - (sum_t m_t * L0[target_t]) / sum(m), with
    L0 = mask_emb @ W (shared logits row at every masked token) and
    S0 = logsumexp(L0).  Only masked tokens contribute.
  - target_t = argmax_n score_tn, score_tn = proj_t . emb_n - 0.5|emb_n|^2.
  - The kernel computes beta-scaled scores directly (beta folded into the
    matmul lhs), per 1024-code block g:
        nb_g   = -max_n beta*score          (DVE reduce, negate=True)
        psum  += beta*delta*L0              (K=1 accum matmul vs row17)
        vsum_g = sum_n exp(beta*v + nb_g)   (ACT exp, bias=nb_g, accum)
    then vtot = sum_g vsum_g * exp(-nb_g - max_g(-nb_g)) ~= exp(beta*delta*
    L0[argmax]); the host takes ln(vtot) (keeps Ln out of the hot loop so
    only one ACT table set is ever loaded mid-kernel).
  - 4096 masked tokens -> 4 tiles x 128 per core; the <=128 leftovers are
    replicated on every core as a "tail" tile where each core scores only
    its own 1/8 of the codebook (per-core block-rotated codebook; argmax is
    column-order invariant) and the host combines the per-core partials.
  - W is shipped fp8e4 and streamed once to build the delta*L0 row (16
    matmuls into col-group-packed PSUM rows, M=32 replication so the row
    escapes PSUM in one full-width ACT copy); S0 comes from that row
    reshaped to [128,64].
"""

import math

import numpy as np

try:
    import concourse.bass as bass  # noqa: F401
except ImportError:  # pragma: no cover
    import sys

    sys.path.insert(0, "/opt/trn_rl_repo")
    import concourse.bass as bass  # noqa: F401

import concourse.mybir as mybir
from concourse import bacc, bass_utils, masks
from concourse.tile import TileContext

F32 = mybir.dt.float32
BF16 = mybir.dt.bfloat16
FP8 = mybir.dt.float8e4

B, T, D, E, N = 16, 512, 256, 16, 8192
NCORES = 8
EPS = 1e-5
DELTA = 1e-2
BETA = 2000.0
NBLK = 8          # 1024-code blocks
BLK = N // NBLK

_CACHE = {}


def _build_bass(NT, use_tail):
    nc = bacc.Bacc(
        "TRN2", target_bir_lowering=False, debug=False, num_devices=NCORES
    )
    NLN = NT + (1 if use_tail else 0)
    xsm = nc.dram_tensor("xsm", [128, NT, D], F32, kind="ExternalInput")
    xst = nc.dram_tensor("xst", [128, D], F32, kind="ExternalInput")
    embb = nc.dram_tensor("embb", [E, N], BF16, kind="ExternalInput")
    qrow = nc.dram_tensor("qrow", [1, N], BF16, kind="ExternalInput")
    ppb = nc.dram_tensor("ppb", [128, 2, E], BF16, kind="ExternalInput")
    b0t = nc.dram_tensor("b0t", [E, 1], F32, kind="ExternalInput")
    mk2 = nc.dram_tensor("mk2", [128, 2, 32], FP8, kind="ExternalInput")
    idin = nc.dram_tensor("idin", [128, 128], BF16, kind="ExternalInput")
    wb = nc.dram_tensor("wb", [NBLK, 128, 2, 1024], FP8, kind="ExternalInput")
    out = nc.dram_tensor("out", [128, NT + 3], F32, kind="ExternalOutput")

    AX = mybir.AxisListType.X
    OP = mybir.AluOpType
    AF = mybir.ActivationFunctionType

    with TileContext(nc) as tc:
        with (
            tc.tile_pool(name="cst", bufs=1) as cst,
            tc.tile_pool(name="xsp", bufs=1) as xsp,
            tc.tile_pool(name="wp", bufs=1) as wp,
            tc.tile_pool(name="lnp", bufs=2) as lnp,
            tc.tile_pool(name="lhp", bufs=NLN) as lhp,
            tc.tile_pool(name="smp", bufs=2) as smp,
            tc.tile_pool(name="ps", bufs=3, space="PSUM") as ps,
            tc.tile_pool(name="psm", bufs=1, space="PSUM") as psm,
        ):
            # ---------------- constants / big DMAs ----------------
            em17 = cst.tile([17, N], BF16)
            row17 = cst.tile([1, N], BF16)
            mk = cst.tile([128, 2, 32], FP8)
            nc.sync.dma_start(mk[:], mk2[:, :, :])
            xall = xsp.tile([128, NT, D], F32)
            xtail = xsp.tile([128, D], F32)
            wres = []
            for g in range(NBLK):
                wt = wp.tile([128, 2, 1024], FP8, tag="wt", name="wt",
                             bufs=NBLK)
                nc.sync.dma_start(wt[:], wb[g, :, :, :])
                wres.append(wt)
                if g == 1:
                    nc.sync.dma_start(xall[:], xsm[:, :, :])
                    if use_tail:
                        nc.sync.dma_start(xtail[:], xst[:, :])
            nc.sync.dma_start(em17[0:16, :], embb[:, :])
            nc.sync.dma_start(em17[16:17, :], qrow[:, :])
            ident = cst.tile([128, 128], BF16)
            nc.sync.dma_start(ident[:], idin[:, :])
            pp = cst.tile([128, 2, E], BF16)
            nc.sync.dma_start(pp[:], ppb[:, :, :])
            b0 = cst.tile([E, 1], F32)
            nc.sync.dma_start(b0[:], b0t[:, :])

            onesb = cst.tile([1, 128], BF16)     # beta row for K=1 accum
            nc.vector.memset(onesb[:], BETA)

            epsb = cst.tile([128, 1], F32)
            nc.vector.memset(epsb[:], EPS)

            etr = cst.tile([128, BLK], BF16)       # exp trash output
            dl0rep = cst.tile([128, 1024], BF16)   # delta*L0, 32x-replicated
            s0t = cst.tile([128, 64], BF16)
            s0acc = cst.tile([128, 1], F32)
            vt_all = cst.tile([128, NT], F32)
            mvall = cst.tile([128, 2 * NLN], F32)
            lnv_all = cst.tile([128, NLN], F32)
            rstd_all = cst.tile([128, NLN], F32)

            # ---------------- LN stats, batched by ACT table set ----------
            for i in range(NLN):
                x_t = xall[:, i, :] if i < NT else xtail[:]
                st6 = lnp.tile([128, 6], F32, tag="st6")
                nc.vector.bn_stats(st6[:], x_t)
                nc.vector.bn_aggr(mvall[:, 2 * i:2 * i + 2], st6[:])
            for i in range(NLN):
                nc.scalar.activation(
                    lnv_all[:, i:i + 1], mvall[:, 2 * i + 1:2 * i + 2],
                    AF.Ln, bias=epsb[:],
                )
            for i in range(NLN):
                nc.scalar.activation(
                    rstd_all[:, i:i + 1], lnv_all[:, i:i + 1],
                    AF.Exp, scale=-0.5,
                )

            def ln_tile(i, early=False):
                """z -> zT -> proj -> beta-scaled lhs [17,128] for tile i."""
                x_t = xall[:, i, :] if i < NT else xtail[:]
                z = lnp.tile([128, D], BF16, tag="z")
                nc.vector.tensor_scalar(
                    z[:], x_t, mvall[:, 2 * i:2 * i + 1],
                    rstd_all[:, i:i + 1], op0=OP.subtract, op1=OP.mult,
                )
                pool, tg = (ps, "pair") if early else (psm, "misc")
                mtz = pool.tile([128, 1024], F32, tag=tg, name="mtz")
                ztp = mtz[:].bitcast(BF16)[:, 0:256]
                for kc in range(2):
                    nc.tensor.transpose(
                        ztp[:, kc * 128:(kc + 1) * 128],
                        z[:, kc * 128:(kc + 1) * 128], ident[:],
                    )
                zt = lnp.tile([128, 2, 128], BF16, tag="ztsb")
                nc.scalar.activation(zt[:, 0, :], ztp[:, 0:128], AF.Copy)
                nc.scalar.activation(zt[:, 1, :], ztp[:, 128:256], AF.Copy)
                mtp = pool.tile([128, 1024], F32, tag=tg, name="mtp")
                ppj = mtp[0:16, 0:128]
                for kc in range(2):
                    nc.tensor.matmul(
                        ppj, pp[:, kc, :], zt[:, kc, :],
                        start=(kc == 0), stop=(kc == 1),
                    )
                lhs = lhp.tile([17, 128], BF16, tag="lhs")
                nc.vector.memset(lhs[:], BETA)   # row16 = beta (q-row coeff)
                nc.vector.tensor_scalar(
                    lhs[0:16, :], ppj, b0[:], BETA, op0=OP.add, op1=OP.mult
                )
                return lhs

            def psl_batch(b):
                """fp8 W chunks 4b..4b+3 -> delta*L0 row cols [4096b:...]."""
                psl = psm.tile([128, 1024], F32, tag="misc", name="psl")[:]
                for cg in range(4):
                    g = 4 * b + cg
                    for h in range(2):
                        hs = slice(h * 512, (h + 1) * 512)
                        for dc in range(2):
                            nc.tensor.matmul(
                                psl[32 * cg:32 * cg + 32, hs],
                                mk[:, dc, :], wres[g][:, dc, hs],
                                start=(dc == 0), stop=(dc == 1),
                                tile_position=(0, 32 * cg),
                            )
                nc.scalar.activation(dl0rep[:, :], psl, AF.Copy, scale=DELTA)
                src = dl0rep[:].rearrange("(c s) j -> c s j", s=32)[:, 0:1, :]
                nc.sync.dma_start(row17[0:1, b * 4096:(b + 1) * 4096], src)

            def mm_score(lhs, g, pool, tg):
                pt = pool.tile([128, BLK], F32, tag=tg, name="pt")
                for h in range(2):
                    hs = slice(g * BLK + h * 512, g * BLK + (h + 1) * 512)
                    nc.tensor.matmul(
                        pt[:, h * 512:(h + 1) * 512], lhs[0:17, :],
                        em17[0:17, hs], start=True, stop=True,
                    )
                return pt

            def block_tail(pt, g, nb, vsum):
                for h in range(2):
                    hs = slice(g * BLK + h * 512, g * BLK + (h + 1) * 512)
                    nc.tensor.matmul(
                        pt[:, h * 512:(h + 1) * 512], onesb[:],
                        row17[0:1, hs], start=False, stop=True,
                        skip_group_check=True,
                    )
                nc.scalar.activation(
                    etr[:], pt[:], AF.Exp, bias=nb[:, g:g + 1],
                    accum_out=vsum[:, g:g + 1],
                )

            def do_tile(lhs, nblk, nb, vsum, lag=3):
                """Software-pipelined: MMdelta/EXP trail MM_s by `lag`
                blocks so no PE instruction waits on an in-flight DVE max.
                Every 4th block borrows the misc PSUM ring for a 4-deep
                effective pipeline."""
                pend = []
                for g in range(nblk):
                    if g % 4 == 3:
                        pt = mm_score(lhs, g, psm, "misc")
                    else:
                        pt = mm_score(lhs, g, ps, "pair")
                    nc.vector.tensor_reduce(
                        nb[:, g:g + 1], pt[:], axis=AX, op=OP.max, negate=True
                    )
                    pend.append((pt, g))
                    if len(pend) > lag:
                        block_tail(*pend.pop(0), nb, vsum)
                for item in pend:
                    block_tail(*item, nb, vsum)

            def combine_tile(i, nb, vsum):
                tmin = smp.tile([128, 1], F32, tag="tmin")
                nc.vector.tensor_reduce(tmin[:], nb[:], axis=AX, op=OP.min)
                wg = smp.tile([128, NBLK], F32, tag="wg")
                nc.scalar.activation(
                    wg[:], nb[:], AF.Exp, scale=-1.0, bias=tmin[:]
                )
                vd = smp.tile([128, NBLK], F32, tag="vd")
                nc.vector.tensor_tensor(vd[:], vsum[:], wg[:], op=OP.mult)
                nc.vector.tensor_reduce(
                    vt_all[:, i:i + 1], vd[:], axis=AX, op=OP.add
                )

            def tile_state():
                nb = smp.tile([128, NBLK], F32, tag="nb")
                vsum = smp.tile([128, NBLK], F32, tag="vsum")
                return nb, vsum

            # ---------------- emission schedule ----------------
            psl_batch(0)                       # row17 cols 0:4096
            lhs_all = {}
            for i in range(NLN):
                lhs_all[i] = ln_tile(i, early=True)
            psl_batch(1)                       # row17 cols 4096:8192

            st0 = tile_state()
            do_tile(lhs_all[0], NBLK, st0[0], st0[1])
            combine_tile(0, st0[0], st0[1])

            # S0 partials: row17 -> [128,64] -> exp-accum
            nc.sync.dma_start(s0t[:], row17[0:1, :])
            nc.scalar.activation(
                etr[:, 0:64], s0t[:], AF.Exp, scale=1.0 / DELTA,
                accum_out=s0acc[:],
            )

            for i in range(1, NT):
                sti = tile_state()
                do_tile(lhs_all[i], NBLK, sti[0], sti[1])
                combine_tile(i, sti[0], sti[1])
                if i == 1 and use_tail:
                    stt = tile_state()
                    do_tile(lhs_all[NT], 1, stt[0], stt[1])
                    nc.sync.dma_start(out[:, NT + 1:NT + 2], stt[0][:, 0:1])
                    nc.sync.dma_start(out[:, NT + 2:NT + 3], stt[1][:, 0:1])
            if use_tail and NT == 1:
                stt = tile_state()
                do_tile(lhs_all[NT], 1, stt[0], stt[1])
                nc.sync.dma_start(out[:, NT + 1:NT + 2], stt[0][:, 0:1])
                nc.sync.dma_start(out[:, NT + 2:NT + 3], stt[1][:, 0:1])

            # outputs
            nc.sync.dma_start(out[:, 0:NT], vt_all[:])
            nc.sync.dma_start(out[:, NT:NT + 1], s0acc[:])

    nc.finalize()
    return nc


def _prep_in_maps(xs, pad_mask, masked_masks, ln_gamma, ln_beta, projection,
                  embeddings, top_n_out, mask_emb):
    import ml_dtypes

    xsf = np.ascontiguousarray(np.asarray(xs, np.float32).reshape(B * T, D))
    pmf = np.asarray(pad_mask).reshape(-1).astype(bool)
    mmf = np.asarray(masked_masks).reshape(-1).astype(bool)
    gamma = np.asarray(ln_gamma, np.float32)
    beta = np.asarray(ln_beta, np.float32)
    proj = np.asarray(projection, np.float32)
    emb = np.asarray(embeddings, np.float32)[0]          # [E, N]
    wmat = np.asarray(top_n_out, np.float32)[0]          # [D, N]
    maske = np.asarray(mask_emb, np.float32)

    sel = np.nonzero(pmf & mmf)[0]
    n = len(sel)
    NT = max(1, -(-max(n - 128, 1) // (NCORES * 128)))
    nmain = min(n, NCORES * 128 * NT)
    L = n - nmain
    assert L <= 128, f"tail overflow: {L}"
    use_tail = L > 0

    main_idx = sel[:nmain]
    xs_cores, m_cores = [], []
    for c in range(NCORES):
        idx = main_idx[c * 128 * NT:(c + 1) * 128 * NT]
        k = len(idx)
        xc = np.zeros((NT * 128, D), np.float32)
        mc = np.zeros((NT * 128,), np.float32)
        if k:
            xc[:k] = xsf[idx]
            mc[:k] = 1.0
        xs_cores.append(
            np.ascontiguousarray(xc.reshape(NT, 128, D).transpose(1, 0, 2))
        )
        m_cores.append(
            np.ascontiguousarray(mc.reshape(NT, 128).transpose(1, 0))
        )

    xt = np.zeros((128, D), np.float32)
    if use_tail:
        xt[:L] = xsf[sel[nmain:]]

    # gamma folded into projection, beta into the projected bias (host fold)
    ppf = (gamma[:, None] * proj).astype(np.float32)     # [D, E]
    b0 = (beta @ proj).astype(np.float32).reshape(E, 1)  # [E, 1]
    ppb = np.ascontiguousarray(
        ppf.reshape(2, 128, E).transpose(1, 0, 2)
    ).astype(ml_dtypes.bfloat16)

    # mk2[p, dc, j] = maske[dc*128 + p], replicated over 32 cols (M=32)
    mk2 = np.ascontiguousarray(
        np.repeat(maske.reshape(2, 128).T[:, :, None], 32, axis=2)
    ).astype(ml_dtypes.float8_e4m3fn)

    in_maps = []
    for c in range(NCORES):
        perm = np.roll(np.arange(N), -c * 1024)
        embP = emb[:, perm]
        wP = wmat[:, perm]
        qrowP = (-0.5 * np.sum(embP.astype(np.float64) ** 2, axis=0)).astype(
            np.float32
        ).reshape(1, N)
        wbP = np.ascontiguousarray(
            wP.reshape(2, 128, NBLK, 1024).transpose(2, 1, 0, 3)
        )
        in_maps.append({
            "xsm": xs_cores[c],
            "xst": xt,
            "idin": np.eye(128, dtype=np.float32).astype(ml_dtypes.bfloat16),
            "embb": np.ascontiguousarray(embP).astype(ml_dtypes.bfloat16),
            "qrow": qrowP.astype(ml_dtypes.bfloat16),
            "ppb": ppb,
            "b0t": b0,
            "mk2": mk2,
            "wb": wbP.astype(ml_dtypes.float8_e4m3fn),
        })
    return in_maps, (NT, use_tail, n, L, m_cores)


def kernel(**inputs) -> np.ndarray:
    in_maps, (NT, use_tail, n, L, m_cores) = _prep_in_maps(**inputs)
    key = (NT, use_tail)
    if key not in _CACHE:
        _CACHE[key] = _build_bass(NT, use_tail)
        _CACHE["nc"] = _CACHE[key]
    nc = _CACHE[key]
    res = bass_utils.run_bass_kernel_spmd(
        nc, in_maps, core_ids=list(range(NCORES))
    )
    num = 0.0
    s0sum = None
    bmt = np.zeros((NCORES, 128), np.float64)
    vst = np.zeros((NCORES, 128), np.float64)
    for c, r in enumerate(res.results):
        o = np.asarray(r["out"], np.float64).reshape(128, NT + 3)
        m = m_cores[c].astype(np.float64)                 # [128, NT]
        num += float((m * np.log(o[:, 0:NT])).sum())
        s0sum = float(o[:, NT].sum())
        bmt[c] = -o[:, NT + 1]                            # beta*bmax
        vst[c] = o[:, NT + 2]
    if L > 0:
        gm = bmt.max(axis=0)
        w = np.exp(bmt - gm[None, :])
        vtot = (vst * w).sum(axis=0)
        num += float(np.log(vtot[:L]).sum())
    s0 = math.log(s0sum)
    loss = np.float32(s0 - num / (BETA * DELTA) / n)
    return np.asarray(loss, np.float32)
